# revision 1
# baseline (speedup 1.0000x reference)
"""MoE layer (top-2 of 8 experts, SwiGLU FFN) on 8 Trainium2 NeuronCores.

Expert-parallel sharding: core e holds expert e's weights (W1/W2/W3 slices).
Host computes the (tiny) router matmul + top-2 dispatch, gathers each
expert's tokens, and ships them transposed so the device kernel is a pure
grouped GEMM:

    h.T = W1e.T @ Xe.T ; g.T = W3e.T @ Xe.T          (contract over D)
    a.T = silu(h.T) * g.T                            (ACT + DVE)
    y.T = W2e.T @ a.T                                (contract over F)

All matmuls run in float32r (full fp32 storage, ~1 cycle/row on the PE at
moving-dim >= 256, ~1e-4 rel err).  Host applies the router probabilities
and scatter-adds the per-expert outputs back into the full [B,S,D] output.

Weights are pre-tiled on the host so every DMA is a contiguous
[128, free] panel:
    w1t/w3t: [ft, ki, (ko f)]  -- lhsT panels for stage 1
    w2t:     [dt, fi, (fo d)]  -- lhsT panels for stage 2
    xt:      [ko, ki, c]       -- tokens transposed, padded to C columns
"""

import numpy as np

import concourse.tile as tile
from concourse import bacc, mybir
from concourse.bass_utils import run_bass_kernel_spmd

N_CORES = 8
P = 128  # SBUF partitions / matmul tile edge

# Results of the most recent device run (for the test harness / profiling).
last_results = None

_NC_CACHE = {}


def _build_nc(ko, ft, dt, nb, blk):
    """Device program: SwiGLU FFN for one expert over C = nb*blk tokens.

    ko = D/128 (stage-1 contraction tiles), ft = F/128 (stage-1 psum tiles),
    dt = D/128 (stage-2 psum tiles), nb = token column blocks of width blk.
    """
    C = nb * blk
    f32 = mybir.dt.float32
    f32r = mybir.dt.float32r
    silu = mybir.ActivationFunctionType.Silu

    nc = bacc.Bacc("TRN2", target_bir_lowering=False, debug=False,
                   num_devices=N_CORES)
    xt_d = nc.dram_tensor("xt", [ko, P, C], f32r, kind="ExternalInput")
    w1_d = nc.dram_tensor("w1t", [ft, P, ko * P], f32r, kind="ExternalInput")
    w3_d = nc.dram_tensor("w3t", [ft, P, ko * P], f32r, kind="ExternalInput")
    w2_d = nc.dram_tensor("w2t", [dt, P, ft * P], f32r, kind="ExternalInput")
    yt_d = nc.dram_tensor("yt", [dt, P, C], f32, kind="ExternalOutput")

    with tile.TileContext(nc) as tc:
        with (
            tc.tile_pool(name="xpool", bufs=1) as xpool,
            tc.tile_pool(name="wpool", bufs=4) as wpool,
            tc.tile_pool(name="w2pool", bufs=3) as w2pool,
            tc.tile_pool(name="actpool", bufs=1) as actpool,
            tc.tile_pool(name="hpool", bufs=2) as hpool,
            tc.tile_pool(name="ypool", bufs=2) as ypool,
            tc.tile_pool(name="psh", bufs=3, space="PSUM") as psh,
            tc.tile_pool(name="psg", bufs=3, space="PSUM") as psg,
            tc.tile_pool(name="psy", bufs=2, space="PSUM") as psy,
        ):
            # first weight panels lead the DMA dispatch order: the first
            # matmul group is gated on them
            panel0 = {}
            for tag, dram in (("w1", w1_d), ("w3", w3_d)):
                sb = wpool.tile([P, ko * P], f32r, tag=tag)
                nc.sync.dma_start(out=sb[:], in_=dram[0])
                panel0[tag] = sb
            x_sb = xpool.tile([P, ko * C], f32r)
            # split across DMA queues so the first matmul isn't gated on one
            # serial 2.4MB transfer
            for k in range(ko):
                nc.sync.dma_start(out=x_sb[:, k * C:(k + 1) * C], in_=xt_d[k])
            # act.T[(ft fi), c] staged for stage 2; one flat tile -- stage 2
            # needs all of stage 1 anyway, so coarse dep tracking is fine.
            act_all = actpool.tile([P, ft * C], f32r)

            for f in range(ft):
                if f == 0:
                    w1_sb, w3_sb = panel0["w1"], panel0["w3"]
                else:
                    w1_sb = wpool.tile([P, ko * P], f32r, tag="w1")
                    nc.sync.dma_start(out=w1_sb[:], in_=w1_d[f])
                    w3_sb = wpool.tile([P, ko * P], f32r, tag="w3")
                    nc.sync.dma_start(out=w3_sb[:], in_=w3_d[f])
                for b in range(nb):
                    ph = psh.tile([P, blk], f32)
                    pg = psg.tile([P, blk], f32)
                    for k in range(ko):
                        rhs = x_sb[:, k * C + b * blk: k * C + (b + 1) * blk]
                        nc.tensor.matmul(ph[:], w1_sb[:, k * P:(k + 1) * P],
                                         rhs, start=(k == 0), stop=(k == ko - 1))
                    for k in range(ko):
                        rhs = x_sb[:, k * C + b * blk: k * C + (b + 1) * blk]
                        nc.tensor.matmul(pg[:], w3_sb[:, k * P:(k + 1) * P],
                                         rhs, start=(k == 0), stop=(k == ko - 1))
                    sh = hpool.tile([P, blk], f32)
                    nc.scalar.activation(sh[:], ph[:], silu)
                    nc.vector.tensor_mul(
                        act_all[:, f * C + b * blk: f * C + (b + 1) * blk],
                        sh[:], pg[:])

            for d in range(dt):
                w2_sb = w2pool.tile([P, ft * P], f32r)
                nc.sync.dma_start(out=w2_sb[:], in_=w2_d[d])
                for b in range(nb):
                    py = psy.tile([P, blk], f32)
                    for f in range(ft):
                        nc.tensor.matmul(
                            py[:], w2_sb[:, f * P:(f + 1) * P],
                            act_all[:, f * C + b * blk: f * C + (b + 1) * blk],
                            start=(f == 0), stop=(f == ft - 1))
                    y_sb = ypool.tile([P, blk], f32)
                    nc.vector.tensor_copy(y_sb[:], py[:])
                    nc.sync.dma_start(out=yt_d[d][:, b * blk:(b + 1) * blk],
                                      in_=y_sb[:])
    nc.compile()
    return nc


def _route(xt, Wr):
    """Replicate the reference's top-2 routing on host (fp32).

    Selection is robust: 2nd/3rd logit gaps are >> fp32 matmul noise.
    Stable argsort on -logits matches jax.lax.top_k tie-breaking
    (lower index first on exact ties).
    """
    logits = xt @ Wr                                     # [T, E] f32
    order = np.argsort(-logits, axis=1, kind="stable")[:, :2]
    v = np.take_along_axis(logits, order, axis=1)
    ex = np.exp(v - v[:, :1])
    probs = ex / ex.sum(axis=1, keepdims=True)           # [T, 2] f32
    return order, probs


def kernel(x, Wr, W1, W2, W3):
    global last_results
    x = np.asarray(x)
    Wr, W1, W2, W3 = (np.asarray(a) for a in (Wr, W1, W2, W3))
    b, s, D = x.shape
    E = Wr.shape[1]
    F = W1.shape[2]
    T = b * s
    assert E == N_CORES
    ko, ft, dt = D // P, F // P, D // P

    xt = np.ascontiguousarray(x.reshape(T, D), dtype=np.float32)
    order, probs = _route(xt, Wr)

    idx = [np.nonzero((order == e).any(axis=1))[0] for e in range(E)]
    maxc = max(len(i) for i in idx)

    # Token columns split into nb blocks of width blk (mult of 4, <=512 for
    # one PSUM bank, >=256 for full-rate fp32r).
    nb = max(2, -(-maxc // 512))
    blk = max(256, -(-maxc // (nb * 4)) * 4)
    C = nb * blk

    key = (ko, ft, dt, nb, blk)
    if key not in _NC_CACHE:
        _NC_CACHE[key] = _build_nc(*key)
    nc = _NC_CACHE[key]

    in_maps = []
    for e in range(E):
        ids = idx[e]
        xe = np.zeros((ko, P, C), dtype=np.float32)
        if len(ids):
            xe[:, :, :len(ids)] = xt[ids].T.reshape(ko, P, len(ids))
        w1t = np.ascontiguousarray(
            W1[e].reshape(ko, P, ft, P).transpose(2, 1, 0, 3)
        ).reshape(ft, P, ko * P)
        w3t = np.ascontiguousarray(
            W3[e].reshape(ko, P, ft, P).transpose(2, 1, 0, 3)
        ).reshape(ft, P, ko * P)
        w2t = np.ascontiguousarray(
            W2[e].reshape(ft, P, dt, P).transpose(2, 1, 0, 3)
        ).reshape(dt, P, ft * P)
        in_maps.append({"xt": xe, "w1t": w1t, "w3t": w3t, "w2t": w2t})

    res = run_bass_kernel_spmd(nc, in_maps, core_ids=list(range(N_CORES)))
    last_results = res

    out = np.zeros((T, D), dtype=np.float64)
    for e in range(E):
        ids = idx[e]
        if not len(ids):
            continue
        ye = res.results[e]["yt"].reshape(D, C)[:, :len(ids)]  # [D, Ne]
        slot = (order[ids] == e).argmax(axis=1)
        pe = probs[ids, slot].astype(np.float64)
        out[ids] += ye.T.astype(np.float64) * pe[:, None]
    return out.astype(np.float32).reshape(b, s, D)

